# revision 6
# baseline (speedup 1.0000x reference)
# Trainium2 Bass kernel for nn_BinaryClassifier (one-hot -> LSTM -> FC).
#
# Data-parallel over batch: 8 sorted sequences per NeuronCore. Per core the
# LSTM runs 2048 sequential steps. Per step, W_hh streams through the PE as
# 64 bf16 [128,128] stationary tiles (~26.6ns each, FWL) against the
# transposed h state ([128 hidden, 8 batch], 33-slot SBUF ring). The input
# projection xg = (W_ih + biases)[token] is precomputed on the HOST and
# DMA-streamed (no on-device one-hot matmuls). Gates accumulate gate-major
# into per-region PSUM tiles (separate tiles per accumulation group: f,i,g
# per half + o per half), injected with xg by identity matmuls.
#
# Step schedule (the recurrence chain, not the weight stream, is the
# bottleneck): f,i,g matmuls of a half run first, then that half's o
# matmuls; tanh(f,i,g) and the DVE c-update overlap the remaining matmul
# stream, so each half's h path is tanh_fig -> c update -> tanh(c) ->
# h mult, with tanh(o) computed off the critical path right after its o
# matmuls. ACT ops are emitted in readiness order (the ACT queue is
# strictly in-order): figA, tuoA, figB, tncA, tuoB, tncB. Sigmoids are
# folded into pre-scaled weights (sigmoid(x) = (tanh(x/2)+1)/2, h stored
# as 2h, c as 2c) so one tanh covers every gate. Whole chunks of h are
# DMAd to DRAM; the host gathers h at t = len-1 and applies the FC during
# unsharding.
import sys
sys.path.insert(0, '/opt/trn_rl_repo')
from contextlib import ExitStack

import numpy as np
import ml_dtypes

import concourse.bass as bass
import concourse.mybir as mybir
from concourse.tile import TileContext
from concourse.bass import ds
from concourse.bass_utils import run_bass_kernel_spmd

F32 = mybir.dt.float32
BF16 = mybir.dt.bfloat16
AF = mybir.ActivationFunctionType
ALU = mybir.AluOpType

H = 512
V = 25
S = 2048
N_CORES = 8
BLOC = 8          # sequences per core
CH = 16           # steps per xg half-chunk
BODY = 2 * CH     # steps per rep
NM = 16           # gate tiles (4H / 128)
NK = 4            # contraction tiles (H / 128)

_TPB_ENGINES = None


def split_multi_waits(nc):
    """walrus in this container supports only ONE sync wait per TPB engine
    instruction; split extra waits onto preceding same-engine NOPs."""
    global _TPB_ENGINES
    if _TPB_ENGINES is None:
        _TPB_ENGINES = {mybir.EngineType.Pool, mybir.EngineType.Activation,
                        mybir.EngineType.PE, mybir.EngineType.DVE,
                        mybir.EngineType.SP}
    ctr = 0
    for fn in nc.m.functions:
        for bb in fn.blocks:
            new = []
            for inst in bb.instructions:
                si = inst.sync_info
                if (si is not None and len(si.on_wait) > 1
                        and inst.engine in _TPB_ENGINES):
                    waits = list(si.on_wait)
                    for w in waits[:-1]:
                        nop = mybir.InstNoOp(name=f"wsplit-{ctr}", ins=[],
                                             outs=[])
                        ctr += 1
                        nop.engine = inst.engine
                        nop.sync_info = mybir.SyncInfo(on_wait=[w],
                                                       on_update=[])
                        new.append(nop)
                    si.on_wait = waits[-1:]
                    inst.sync_info = si
                new.append(inst)
            bb.instructions = new


def _host_prep(tokens, lengths, W_ih, W_hh, b_ih, b_hh, fc_w, fc_b):
    """Full inputs -> list of per-core input dicts (numpy).

    Column-block numbering (16 blocks of 8 batch cols): blocks 0-5 half-A
    f,i,g_cell; 6-11 half-B f,i,g_cell; 12-13 half-A o; 14-15 half-B o.
    Hidden slice j = jh*2 + jl (jh = half, jl = slice within half)."""
    bf = ml_dtypes.bfloat16
    order = np.argsort(-lengths.astype(np.int64), kind='stable')
    toks = np.asarray(tokens)[order]
    lens = np.asarray(lengths)[order].astype(np.int64)

    # rows of W_* are 4H in torch gate order i,f,g,o; our gate order:
    # f, i, g_cell, o (o last: its matmuls run after the c-path has started)
    perm = np.concatenate([np.arange(1 * H, 2 * H),      # f
                           np.arange(0 * H, 1 * H),      # i
                           np.arange(2 * H, 3 * H),      # g_cell
                           np.arange(3 * H, 4 * H)])     # o
    Whh_p = np.asarray(W_hh)[perm].astype(np.float32)    # [4H, H]
    E_p = (np.asarray(W_ih) + np.asarray(b_ih)[:, None]
           + np.asarray(b_hh)[:, None])[perm].astype(np.float32)
    # sigmoid(x) = (tanh(x/2)+1)/2: pre-halve i,f,o gate rows so one tanh
    # covers all gates; h is stored as 2h, so W_hh is halved again.
    ifo = np.ones(4 * H, bool)
    ifo[2 * H:3 * H] = False                              # g_cell rows not
    Whh_p[ifo] *= 0.5
    E_p[ifo] *= 0.5
    Whh_p *= 0.5                                          # h2 = 2h convention

    # w_lhsT: [128, NK*NM*128], tile (k, m2) at cols (k*NM+m2)*128
    # L: embedding lookup table [V, NM, 128] in m2 order
    w = np.zeros((128, NK * NM * 128), np.float32)
    L = np.zeros((V, NM, 128), np.float32)
    for jh in range(2):
        for g in range(4):
            for jl in range(2):
                m2 = (jh * 6 + g * 2 + jl) if g < 3 else (12 + jh * 2 + jl)
                j = jh * 2 + jl
                rows = slice(g * H + j * 128, g * H + j * 128 + 128)
                for k in range(NK):
                    blk = Whh_p[rows, k * 128:(k + 1) * 128]
                    w[:, (k * NM + m2) * 128:(k * NM + m2 + 1) * 128] = blk.T
                L[:, m2, :] = E_p[rows, :].T
    Lb = L.astype(bf)
    wb = w.astype(bf)
    ident = np.eye(128, dtype=np.float32).astype(bf)

    per_core = []
    for ci in range(N_CORES):
        bs = slice(ci * BLOC, (ci + 1) * BLOC)
        t_c = toks[bs]                                    # [8, S]
        gat = Lb[t_c]                                     # [8, S, NM, 128]
        # xg cols: t*128 + m2*8 + b ; plus one rep of zero padding for the
        # loop's last prefetch
        xg = np.zeros((128, (S + BODY) * NM * BLOC), bf)
        xg[:, 0:S * NM * BLOC] = np.ascontiguousarray(
            np.transpose(gat, (3, 1, 2, 0))).reshape(128, S * NM * BLOC)
        per_core.append({
            "ident": ident,
            "w_lhsT": wb,
            "xg": xg,
        })
    return per_core, order


def _build_nc():
    assert S % BODY == 0
    ITERS = S // BODY
    nc = bass.Bass("TRN2", target_bir_lowering=False, debug=False,
                   num_devices=N_CORES)
    DT = BF16
    w_d = nc.dram_tensor("w_lhsT", [128, NK * NM * 128], DT,
                         kind="ExternalInput").ap()
    xg_d = nc.dram_tensor("xg", [128, (S + BODY) * NM * BLOC], DT,
                          kind="ExternalInput").ap()
    id_d = nc.dram_tensor("ident", [128, 128], DT, kind="ExternalInput").ap()
    hd_d = nc.dram_tensor("hdump", [128, S * 32], BF16,
                          kind="ExternalOutput").ap()

    with TileContext(nc) as tc, ExitStack() as ctx:
        const = ctx.enter_context(tc.tile_pool(name="const", bufs=1))
        state = ctx.enter_context(tc.tile_pool(name="state", bufs=1))
        scr = ctx.enter_context(tc.tile_pool(name="scr", bufs=4))

        w_sb = const.tile([128, NK * NM * 128], DT, tag="w")
        ident = const.tile([128, 128], DT, tag="ident")
        nc.sync.dma_start(out=w_sb[:], in_=w_d[:])
        nc.sync.dma_start(out=ident[:], in_=id_d[:])

        # h ring: body step i reads slot i, writes slot i+1 (33 slots);
        # slot 32 is copied back to slot 0 at body end. Whole chunks of h
        # are DMAd to DRAM so the host can gather h at t = len-1.
        hring = state.tile([128, 33 * 32], DT, tag="hring")
        c_st = state.tile([128, 32], F32, tag="c")
        nc.vector.memset(hring[:, 0:32], 0)
        nc.vector.memset(c_st[:], 0)

        # xg double buffer: rep r consumes one named buffer while the next
        # rep's chunk DMAs into the other (manual alternation, like hring)
        xgA = state.tile([128, BODY * 128], DT, tag="xgA")
        xgB = state.tile([128, BODY * 128], DT, tag="xgB")

        with tc.tile_pool(name="psum", bufs=1, space="PSUM") as psum:
            gpF = [psum.tile([128, 48], F32, name=f"gpF{jh}", tag=f"gpF{jh}")
                   for jh in range(2)]
            gpO = [psum.tile([128, 16], F32, name=f"gpO{jh}", tag=f"gpO{jh}")
                   for jh in range(2)]

            def fig_mms(jh, xgc, xcol, hT):
                # f,i,g matmuls for one half
                X = gpF[jh]
                lo = jh * 48
                nc.tensor.matmul(
                    X[:], ident[:], xgc[:, xcol + lo:xcol + lo + 48],
                    start=True, stop=False)
                for k in range(NK):
                    for g in range(3):
                        for jl in range(2):
                            m2 = jh * 6 + g * 2 + jl
                            last = (k == 3 and g == 2 and jl == 1)
                            o0 = g * 16 + jl * 8
                            nc.tensor.matmul(
                                X[:, o0:o0 + 8],
                                w_sb[:, (k * NM + m2) * 128:
                                     (k * NM + m2 + 1) * 128],
                                hT[:, k * 8:(k + 1) * 8],
                                start=False, stop=last)

            def o_mms(jh, xgc, xcol, hT):
                # o matmuls for one half
                X = gpO[jh]
                lo = 96 + jh * 16
                nc.tensor.matmul(
                    X[:], ident[:], xgc[:, xcol + lo:xcol + lo + 16],
                    start=True, stop=False)
                for k in range(NK):
                    for jl in range(2):
                        m2 = 12 + jh * 2 + jl
                        last = (k == 3 and jl == 1)
                        nc.tensor.matmul(
                            X[:, jl * 8:jl * 8 + 8],
                            w_sb[:, (k * NM + m2) * 128:
                                 (k * NM + m2 + 1) * 128],
                            hT[:, k * 8:(k + 1) * 8],
                            start=False, stop=last)

            def fig_tanh(jh):
                tfig = scr.tile([128, 48], F32, name=f"tf{jh}",
                                tag=f"tfig{jh}")
                nc.scalar.activation(tfig[:], gpF[jh][:], AF.Tanh)
                return tfig

            def c_dve(jh, tfig):
                # c-update from f,i,g tanh; DVE only (t1 first: the cs op
                # depends on the later-issued t2, minimizing chain depth)
                cs = c_st[:, jh * 16:(jh + 1) * 16]
                t1 = scr.tile([128, 16], F32, name=f"t1_{jh}",
                              tag=f"t1_{jh}")
                t2 = scr.tile([128, 16], F32, name=f"t2_{jh}",
                              tag=f"t2_{jh}")
                nc.vector.scalar_tensor_tensor(
                    t1[:], tfig[:, 16:32], 1.0, tfig[:, 32:48],
                    op0=ALU.add, op1=ALU.mult)
                nc.vector.scalar_tensor_tensor(
                    t2[:], tfig[:, 0:16], 1.0, cs,
                    op0=ALU.add, op1=ALU.mult)
                nc.vector.scalar_tensor_tensor(
                    cs, t2[:], 0.5, t1[:], op0=ALU.mult, op1=ALU.add)
                return cs

            def step(sc, xgc, coff, hT, hTn):
                xcol = coff + sc * 128
                # per half: f,i,g matmuls then o matmuls; tanh(o) fires
                # right after each half's o stream so the h path is only
                # tanh_fig -> c update -> tanh(c) -> h mult. ACT ops are
                # emitted in readiness order (ACT executes in-order):
                # figA, tuoA, figB, tncA, tuoB, tncB.
                fig_mms(0, xgc, xcol, hT)
                tfA = fig_tanh(0)
                cA = c_dve(0, tfA)
                o_mms(0, xgc, xcol, hT)
                tuoA = scr.tile([128, 16], F32, name="tuoA", tag="tuoA")
                nc.scalar.activation(tuoA[:], gpO[0][:], AF.Tanh)
                fig_mms(1, xgc, xcol, hT)
                tfB = fig_tanh(1)
                tncA = scr.tile([128, 16], F32, name="tncA", tag="tncA")
                nc.scalar.activation(tncA[:], cA, AF.Tanh, scale=0.5)
                cB = c_dve(1, tfB)
                o_mms(1, xgc, xcol, hT)
                tuoB = scr.tile([128, 16], F32, name="tuoB", tag="tuoB")
                nc.scalar.activation(tuoB[:], gpO[1][:], AF.Tanh)
                tncB = scr.tile([128, 16], F32, name="tncB", tag="tncB")
                nc.scalar.activation(tncB[:], cB, AF.Tanh, scale=0.5)
                nc.vector.scalar_tensor_tensor(
                    hTn[:, 0:8], tuoA[:, 0:8], 1.0, tncA[:, 0:8],
                    op0=ALU.add, op1=ALU.mult)
                nc.vector.scalar_tensor_tensor(
                    hTn[:, 8:16], tuoA[:, 8:16], 1.0, tncA[:, 8:16],
                    op0=ALU.add, op1=ALU.mult)
                nc.vector.scalar_tensor_tensor(
                    hTn[:, 16:32], tuoB[:], 1.0, tncB[:],
                    op0=ALU.add, op1=ALU.mult)

            # preload xg for rep 0
            nc.sync.dma_start(out=xgA[:], in_=xg_d[:, 0:BODY * 128])

            with tc.For_i(0, ITERS // 4, 1,
                          hint_engines=(mybir.EngineType.PE,)) as iv:
                for rep in range(4):
                    prev, nxt = (xgA, xgB) if rep % 2 == 0 else (xgB, xgA)
                    nc.sync.dma_start(
                        out=nxt[:],
                        in_=xg_d[:, ds(iv * (4 * BODY * 128)
                                       + (rep + 1) * (BODY * 128),
                                       BODY * 128)])
                    rbase = (iv * 4 + rep) * (BODY * 32)
                    for sc in range(CH):
                        a = hring[:, sc * 32:(sc + 1) * 32]
                        b = hring[:, (sc + 1) * 32:(sc + 2) * 32]
                        step(sc, prev, 0, a, b)
                    nc.sync.dma_start(
                        out=hd_d[:, ds(rbase, CH * 32)],
                        in_=hring[:, 32:(CH + 1) * 32])
                    for sc in range(CH):
                        a = hring[:, (CH + sc) * 32:(CH + sc + 1) * 32]
                        b = hring[:, (CH + sc + 1) * 32:(CH + sc + 2) * 32]
                        step(sc, prev, CH * 128, a, b)
                    nc.sync.dma_start(
                        out=hd_d[:, ds(rbase + CH * 32, CH * 32)],
                        in_=hring[:, (CH + 1) * 32:(2 * CH + 1) * 32])
                    nc.gpsimd.tensor_copy(
                        hring[:, 0:32],
                        hring[:, 32 * CH * 2:32 * CH * 2 + 32])

    split_multi_waits(nc)
    return nc


def _gather_out(results, lens_sorted, fc_w, fc_b):
    fcw = np.asarray(fc_w, np.float32)[0]
    fcb = float(np.asarray(fc_b, np.float32)[0])
    out = np.zeros((N_CORES * BLOC, 1), np.float32)
    for ci in range(N_CORES):
        hd = results[ci]["hdump"]
        for b in range(BLOC):
            t = int(lens_sorted[ci * BLOC + b]) - 1
            h2 = np.concatenate(
                [hd[:, t * 32 + k * 8 + b].astype(np.float32)
                 for k in range(4)])
            out[ci * BLOC + b, 0] = 0.5 * float(np.dot(fcw, h2)) + fcb
    return out


_NC_CACHE = None


def kernel(tokens, lengths, W_ih, W_hh, b_ih, b_hh, fc_w, fc_b):
    global _NC_CACHE
    per_core, order = _host_prep(tokens, lengths, W_ih, W_hh, b_ih, b_hh,
                                 fc_w, fc_b)
    if _NC_CACHE is None:
        _NC_CACHE = _build_nc()
    res = run_bass_kernel_spmd(_NC_CACHE, per_core,
                               core_ids=list(range(N_CORES)))
    # reference returns outputs in sorted (desc length) order; shard ci
    # holds sorted ranks ci*8..ci*8+7, so this is already sorted order
    lens_sorted = np.asarray(lengths).astype(np.int64)[order]
    return _gather_out(res.results, lens_sorted, fc_w, fc_b)
